# revision 17
# baseline (speedup 1.0000x reference)
"""Trainium2 Bass kernel for nn_BasicTransformerBlock_18657337934637.

Sparse-attention transformer block:
  q/k/v = hidden @ W* + b*        (2304 -> 2304, 24 heads x 96)
  RoPE3D on q, k
  sparse-1d grouping (SPARSE_N=4): token t -> group t%4, 1024 tokens/group
  softmax attention within each (group, head)
  out = attn @ wo + bo

Distribution over 8 NeuronCores:
  Launch 1 (head-parallel): core c computes heads 3c..3c+2 end-to-end through
    attention.  Host pre-transposes hidden to hT [2304, 4096] in grouped token
    order, so groups are contiguous 1024-token spans and the QKV matmuls need
    no on-device transpose of the activations.  Per (group, head): scores are
    computed transposed [k, q] so softmax-exp sums and the P@V contraction both
    run with k on the partition axis; an all-ones column appended to v yields
    the softmax denominator for free in the same matmul; exp skips the max
    subtraction (scores are O(5), fp32 exp is safe).  Output: un-normalized
    attn^T + denominator row, [3, 97, 4096] per core; the host divides.
  Host: gather heads -> attnT [2304, 4096], undo token permutation.
  Launch 2 (token x outdim parallel): core (i, j) computes
    out[i*1024:(i+1)*1024, j*1152:(j+1)*1152]^T = wo_j^T @ attnT_i
    (output kept transposed so the weight stays stationary on the PE).

Matmuls run as float32r (full fp32 storage; TensorE single-pass mode, 1 row/cyc
for moving dim >= 256).  Set KERNEL_MM_DT=f32 to fall back to exact-fp32
two-pass matmuls.
"""
import os
import numpy as np

HEADS = 24
HD = 96
SPN = 4
S = 4096
DIM = 2304
KC = DIM // 128            # 18 contraction chunks
HPC = 3                    # heads per core
CW = HPC * HD              # 288 columns per core
G = S // SPN               # 1024 tokens per group
TB = 256                   # hT dma block (tokens)
NB = S // TB               # 16 blocks
SCALE = 1.0 / float(np.sqrt(HD))

_CACHE = {}
LAST_RESULTS = []          # test harness introspection


def _mm_dt():
    import concourse.mybir as mybir
    return (mybir.dt.float32 if os.environ.get("KERNEL_MM_DT") == "f32"
            else mybir.dt.float32r)


def _build_launch1():
    import concourse.mybir as mybir
    import concourse.tile as tile
    from concourse import bacc
    from concourse.masks import make_identity

    f32 = mybir.dt.float32
    mm = _mm_dt()
    Exp = mybir.ActivationFunctionType.Exp
    MUL = mybir.AluOpType.mult
    ADD = mybir.AluOpType.add
    nc = bacc.Bacc("TRN2", target_bir_lowering=False, debug=False)

    # all inputs host-pre-tiled to the exact SBUF layouts -> every DMA is a
    # plain 2D copy with multi-KB contiguous rows (full HBM bandwidth)
    hT_d = nc.dram_tensor("hT", [NB, 128, KC * TB], mm,
                          kind="ExternalInput").ap()
    w_d = {n: nc.dram_tensor(n, [128, KC * CW], mm, kind="ExternalInput").ap()
           for n in ("wq", "wk", "wv")}
    b_d = {n: nc.dram_tensor(n, [1, CW], f32, kind="ExternalInput").ap()
           for n in ("bq", "bk", "bv")}
    A_d = nc.dram_tensor("A", [NB, 128, 2 * CW], f32, kind="ExternalInput").ap()
    B_d = nc.dram_tensor("B", [NB, 128, 2 * CW], f32, kind="ExternalInput").ap()
    bvi_d = nc.dram_tensor("bvi", [1, HPC * (HD + 1)], f32,
                           kind="ExternalInput").ap()
    outN_d = nc.dram_tensor("outN", [HPC, HD + 1, S], f32,
                            kind="ExternalOutput").ap()

    with tile.TileContext(nc) as tc:
        with (
            tc.tile_pool(name="singles", bufs=1) as singles,
            tc.tile_pool(name="hp", bufs=2) as hp,
            tc.tile_pool(name="rp", bufs=3) as rp,
            tc.tile_pool(name="qkp", bufs=3) as qkp,
            tc.tile_pool(name="qrp", bufs=3) as qrp,
            tc.tile_pool(name="vp", bufs=16) as vp,
            tc.tile_pool(name="qtp", bufs=2) as qtp,
            tc.tile_pool(name="ktp", bufs=2) as ktp,
            tc.tile_pool(name="ep", bufs=3) as ep,
            tc.tile_pool(name="op", bufs=3) as op,
            tc.tile_pool(name="ppq", bufs=3, space="PSUM") as ppq,
            tc.tile_pool(name="ppt", bufs=1, space="PSUM") as ppt,
            tc.tile_pool(name="pps", bufs=2, space="PSUM") as pps,
            tc.tile_pool(name="ppv", bufs=2, space="PSUM") as ppv,
        ):
            ident = singles.tile([128, 128], f32, tag="ident", name="ident")
            make_identity(nc, ident)
            # prefetch block 0 activations FIRST so the PE can start as soon
            # as the first weight chunk-group lands
            _pref = {}

            def fetch_blk(blk):
                ht = hp.tile([128, KC * TB], mm, tag="ht", name=f"ht{blk}")
                half = (KC // 2) * TB
                nc.sync.dma_start(ht[:, :half], hT_d[blk][:, :half])
                nc.scalar.dma_start(ht[:, half:], hT_d[blk][:, half:])
                a_t = rp.tile([128, 2 * CW], f32, tag="a", name=f"a{blk}")
                nc.scalar.dma_start(a_t, A_d[blk])
                b_t = rp.tile([128, 2 * CW], f32, tag="b", name=f"b{blk}")
                nc.scalar.dma_start(b_t, B_d[blk])
                return ht, a_t, b_t

            _pref[0] = fetch_blk(0)
            # weights in 3 chunk-groups, interleaved across both HWDGE
            # engines, so the first projection matmuls start after ~1/3 of
            # one weight instead of after all three full weights
            WG = 3
            w_grp = {n: [] for n in ("wq", "wk", "wv")}
            b_sb = {}
            for gi in range(WG):
                for wi, n in enumerate(("wq", "wk", "wv")):
                    t = singles.tile([128, (KC // WG) * CW], mm,
                                     tag=f"{n}_sb{gi}", name=f"{n}_sb{gi}")
                    eng = nc.sync if (gi + wi) % 2 == 0 else nc.scalar
                    eng.dma_start(
                        t, w_d[n][:, gi * (KC // WG) * CW:
                                  (gi + 1) * (KC // WG) * CW])
                    w_grp[n].append(t.rearrange("p (k c) -> p k c", k=KC // WG))
            w_sb = {n: None for n in w_grp}

            class _WView:
                def __init__(self, grps):
                    self.grps = grps
                def __getitem__(self, key):
                    _, kc, cs = key
                    return self.grps[kc // (KC // WG)][:, kc % (KC // WG), cs]
            w_sb = {n: _WView(g) for n, g in w_grp.items()}
            for n in ("bq", "bk", "bv"):
                t = singles.tile([128, CW], f32, tag=f"{n}_sb", name=f"{n}_sb")
                nc.gpsimd.dma_start(out=t, in_=b_d[n].to_broadcast([128, CW]))
                b_sb[n] = t
            ones3 = singles.tile([128, HPC], f32, tag="ones3", name="ones3")
            nc.vector.memset(ones3, 1.0)
            bvi_sb = singles.tile([128, HPC * (HD + 1)], f32, tag="bvi",
                                  name="bvi_sb")
            nc.gpsimd.dma_start(out=bvi_sb,
                                in_=bvi_d.to_broadcast([128, HPC * (HD + 1)]))

            qT, kT, vt = {}, {}, {}
            pending = []   # attention instances awaiting emission

            def attn_instance(g, h, qh):
                """scoresT -> exp -> PV for one (group, head, query-half),
                software-pipelined over the 8 key chunks."""
                pv = ppv.tile([HD + 1, 512], f32, tag="pv",
                              name=f"pv{g}_{h}_{qh}")
                qs = qT[g][:, h * G + qh * 512:h * G + (qh + 1) * 512]

                def exp_pv(kc, st):
                    ex = ep.tile([128, 512], mm, tag="ex",
                                 name=f"ex{g}_{h}_{qh}_{kc}")
                    nc.scalar.activation(ex, st, Exp, scale=SCALE)
                    nc.tensor.matmul(
                        pv, vt[(g, kc)][:, h * 97:(h + 1) * 97], ex,
                        start=(kc == 0), stop=(kc == 7))

                sts = []
                for kc in range(8):
                    st = pps.tile([128, 512], f32, tag="st",
                                  name=f"st{g}_{h}_{qh}_{kc}")
                    nc.tensor.matmul(
                        st, kT[g][:, h * G + kc * 128:h * G + (kc + 1) * 128],
                        qs, start=True, stop=True)
                    sts.append(st)
                    if kc >= 1:
                        exp_pv(kc - 1, sts[kc - 1])
                exp_pv(7, sts[7])
                ot = op.tile([HD + 1, 512], f32, tag="ot",
                             name=f"ot{g}_{h}_{qh}")
                nc.scalar.copy(ot, pv)
                nc.scalar.dma_start(
                    outN_d[h, :, g * G + qh * 512:g * G + (qh + 1) * 512], ot)

            for blk in range(NB):
                g = blk // 4
                if blk % 4 == 0:
                    qT[g] = qtp.tile([HD, HPC * G], mm, tag="qT",
                                     name=f"qT{g}")
                    kT[g] = ktp.tile([HD, HPC * G], mm, tag="kT",
                                     name=f"kT{g}")
                ht, a_t, b_t = _pref.pop(blk) if blk in _pref else fetch_blk(blk)
                htv = ht.rearrange("p (k t) -> p k t", k=KC)

                for sub in range(2):
                    tb = blk * 2 + sub
                    col = (tb % 8) * 128
                    a_s = a_t[:, sub * CW:(sub + 1) * CW]
                    b_s = b_t[:, sub * CW:(sub + 1) * CW]
                    # chunk-outer QKV: one stationary hT load serves 3 matmuls
                    ps = {d: ppq.tile([128, CW], f32, tag="ps",
                                      name=f"ps_{d}{tb}")
                          for d in ("q", "k", "v")}
                    for kc in range(KC):
                        lhs = htv[:, kc, sub * 128:(sub + 1) * 128]
                        for n, d in (("wq", "q"), ("wk", "k"), ("wv", "v")):
                            nc.tensor.matmul(
                                ps[d], lhs,
                                w_sb[n][:, kc, slice(None)],
                                start=(kc == 0), stop=(kc == KC - 1))
                    # V: bias add + interleaved ones column, cast to mm
                    v_t = vp.tile([128, HPC * (HD + 1)], mm, tag="v",
                                  name=f"v{tb}")
                    nc.vector.tensor_tensor(
                        v_t.rearrange("p (h c) -> p h c", h=HPC)[:, :, 0:96],
                        ps["v"].rearrange("p (h c) -> p h c", h=HPC),
                        bvi_sb.rearrange("p (h c) -> p h c", h=HPC)[:, :, 0:96],
                        ADD)
                    nc.vector.tensor_copy(
                        v_t.rearrange("p (h c) -> p h c", h=HPC)[:, :, 96:97],
                        ones3.rearrange("p (h c) -> p h c", h=HPC))
                    vt[(g, tb % 8)] = v_t
                    # Q, K: bias, rope, transpose per head
                    for n, d in (("bq", "q"), ("bk", "k")):
                        q_sb = qkp.tile([128, CW], f32, tag=f"{d}sb",
                                        name=f"{d}sb{tb}")
                        nc.vector.tensor_tensor(q_sb, ps[d], b_sb[n], ADD)
                        shf = qkp.tile([128, CW], f32, tag="shf",
                                       name=f"shf_{d}{tb}")
                        qv = q_sb.rearrange("p (h c u f) -> p h c u f",
                                            h=3, c=3, u=2)
                        sv = shf.rearrange("p (h c u f) -> p h c u f",
                                           h=3, c=3, u=2)
                        nc.vector.tensor_copy(sv[:, :, :, 0:1, :],
                                              qv[:, :, :, 1:2, :])
                        nc.vector.tensor_copy(sv[:, :, :, 1:2, :],
                                              qv[:, :, :, 0:1, :])
                        qr = qrp.tile([128, CW], f32, tag="qr",
                                      name=f"qr_{d}{tb}")
                        nc.vector.tensor_tensor(shf, shf, b_s, MUL)
                        nc.vector.tensor_tensor(q_sb, q_sb, a_s, MUL)
                        nc.vector.tensor_tensor(qr, q_sb, shf, ADD)
                        dst = qT if d == "q" else kT
                        pt3 = ppt.tile([HD, HPC * 128], f32, tag="pt",
                                       name=f"pt_{d}{tb}")
                        for h in range(HPC):
                            nc.tensor.transpose(
                                pt3[:, h * 128:(h + 1) * 128],
                                qr[:, h * 96:(h + 1) * 96], ident)
                        nc.scalar.copy(
                            dst[g].rearrange("d (h t) -> d h t", h=HPC)
                            [:, :, col:col + 128],
                            pt3.rearrange("d (h t) -> d h t", h=HPC))
                    # drain one pending attention instance per sub-tile
                    if pending:
                        attn_instance(*pending.pop(0))
                if blk % 4 == 3:
                    pending.extend((g, h, qh)
                                   for h in range(HPC) for qh in range(2))
            while pending:
                attn_instance(*pending.pop(0))
    nc.compile()
    return nc


def _build_launch2():
    import concourse.mybir as mybir
    import concourse.tile as tile
    from concourse import bacc

    f32 = mybir.dt.float32
    bf16 = os.environ.get("KERNEL_L2_BF16", "1") == "1"
    mm = mybir.dt.bfloat16 if bf16 else _mm_dt()
    TOK = 1024           # tokens per core
    NW = 1152            # outdims per core
    MB = NW // 128       # 9 outdim blocks
    nc = bacc.Bacc("TRN2", target_bir_lowering=False, debug=False)

    at_d = nc.dram_tensor("attnT", [DIM, TOK], mm, kind="ExternalInput").ap()
    wo_d = nc.dram_tensor("woj", [DIM, NW], mm, kind="ExternalInput").ap()
    bo_d = nc.dram_tensor("boj", [1, NW], f32, kind="ExternalInput").ap()
    # transposed output [outdim, tok]; host transposes back
    out_d = nc.dram_tensor("out", [NW, TOK], f32, kind="ExternalOutput").ap()

    with tile.TileContext(nc) as tc:
        ats, wos = [], []
        with (
            tc.tile_pool(name="singles2", bufs=1) as singles,
            tc.tile_pool(name="atp", bufs=KC) as atp,
            tc.tile_pool(name="wop", bufs=KC) as wop,
            tc.tile_pool(name="outp", bufs=4) as outp,
            tc.tile_pool(name="psp", bufs=8, space="PSUM") as psp,
        ):
            bo_sb = singles.tile([128, MB], f32, tag="bo_sb", name="bo_sb")
            nc.sync.dma_start(bo_sb,
                              bo_d.rearrange("a (m p) -> p (a m)", p=128))
            for kc in range(KC):
                a = atp.tile([128, TOK], mm, tag="at", name=f"at{kc}")
                nc.sync.dma_start(a, at_d[kc * 128:(kc + 1) * 128, :])
                ats.append(a)
                w = wop.tile([128, NW], mm, tag="wo", name=f"wo{kc}")
                nc.sync.dma_start(w, wo_d[kc * 128:(kc + 1) * 128, :])
                wos.append(w)
            # chunk-outer accumulation over groups of 4 outdim blocks
            # (8 psum banks per group) so the PE tracks the DMA feed instead
            # of serializing behind it.
            units = [(mb, th) for mb in range(MB) for th in range(2)]
            ots = {}
            for base in range(0, len(units), 8):
                grp = units[base:base + 8]
                pss = {}
                for mb, th in grp:
                    pss[(mb, th)] = psp.tile([128, 512], f32, tag="ps",
                                             name=f"ps{mb}_{th}")
                for kc in range(KC):
                    for mb, th in grp:
                        nc.tensor.matmul(
                            pss[(mb, th)], wos[kc][:, mb * 128:(mb + 1) * 128],
                            ats[kc][:, th * 512:(th + 1) * 512],
                            start=(kc == 0), stop=(kc == KC - 1))
                for mb, th in grp:
                    if mb not in ots:
                        ots[mb] = outp.tile([128, TOK], f32, tag="ot",
                                            name=f"ot{mb}")
                    nc.vector.tensor_scalar_add(
                        ots[mb][:, th * 512:(th + 1) * 512], pss[(mb, th)],
                        bo_sb[:, mb:mb + 1])
                    if th == 1:
                        nc.sync.dma_start(out_d[mb * 128:(mb + 1) * 128, :],
                                          ots[mb])
    nc.compile()
    return nc


def _get(name, builder):
    if name not in _CACHE:
        _CACHE[name] = builder()
    return _CACHE[name]


def _rope_tables(frame, height, width):
    t = np.repeat(np.arange(frame), height * width)
    y = np.tile(np.repeat(np.arange(height), width), frame)
    x = np.tile(np.arange(width), frame * height)
    D = HD // 3
    A = np.empty((S, HD), np.float32)
    B = np.empty((S, HD), np.float32)
    for i, pos in enumerate((t, y, x)):
        inv = 1.0 / (10000.0 ** (np.arange(0, D, 2, dtype=np.float32) / D))
        f = pos[:, None].astype(np.float32) * inv[None, :]
        A[:, i * D:i * D + 16] = np.cos(f)
        A[:, i * D + 16:(i + 1) * D] = np.cos(f)
        B[:, i * D:i * D + 16] = -np.sin(f)
        B[:, i * D + 16:(i + 1) * D] = np.sin(f)
    return A, B


def _tile_hT(hT):
    # [2304, 4096] -> [NB, 128, KC*TB]: blk-major, partition-major, then
    # (chunk, token) contiguous per partition
    return np.ascontiguousarray(
        hT.reshape(KC, 128, NB, TB).transpose(2, 1, 0, 3).reshape(
            NB, 128, KC * TB))


def _tile_w(w):
    # [2304, CW] -> [128, KC*CW]
    return np.ascontiguousarray(
        w.reshape(KC, 128, CW).transpose(1, 0, 2).reshape(128, KC * CW))


def _tile_rope(a):
    # [4096, 288] (pre-tripled) -> [NB, 128, 2*288]
    return np.ascontiguousarray(
        a.reshape(NB, 2, 128, CW).transpose(0, 2, 1, 3).reshape(
            NB, 128, 2 * CW))


def kernel(hidden_states, wq, bq, wk, bk, wv, bv, wo, bo, frame, height, width):
    from concourse import bass_utils

    f, hh, ww = int(frame), int(height), int(width)
    hs = np.asarray(hidden_states, dtype=np.float32)
    assert hs.shape == (1, S, DIM) and f * hh * ww == S
    wq, wk, wv, wo = (np.asarray(a, np.float32) for a in (wq, wk, wv, wo))
    bq, bk, bv, bo = (np.asarray(a, np.float32) for a in (bq, bk, bv, bo))

    perm = np.concatenate([np.arange(k, S, SPN) for k in range(SPN)])
    A, B = _rope_tables(f, hh, ww)
    A = _tile_rope(np.tile(A[perm], (1, HPC)))
    B = _tile_rope(np.tile(B[perm], (1, HPC)))
    hT = _tile_hT(hs[0].T[:, perm])

    nc1 = _get("l1", _build_launch1)
    in1 = []
    for c in range(8):
        sl = slice(c * CW, (c + 1) * CW)
        in1.append({
            "hT": hT,
            "wq": _tile_w(wq[:, sl]),
            "wk": _tile_w(wk[:, sl]),
            "wv": _tile_w(wv[:, sl]),
            "bq": np.ascontiguousarray(bq[sl]).reshape(1, CW),
            "bk": np.ascontiguousarray(bk[sl]).reshape(1, CW),
            "bv": np.ascontiguousarray(bv[sl]).reshape(1, CW),
            "bvi": np.concatenate(
                [np.concatenate([bv[sl][h * HD:(h + 1) * HD], [0.0]])
                 for h in range(HPC)]).astype(np.float32).reshape(1, -1),
            "A": A, "B": B,
        })
    td = os.environ.get("KERNEL_TRACE_DIR")
    if td:
        os.makedirs(td + "/l1", exist_ok=True)
        for f in os.listdir(td + "/l1"):
            os.unlink(td + "/l1/" + f)
    res1 = bass_utils.run_bass_kernel_spmd(
        nc1, in1, core_ids=list(range(8)),
        tmpdir=(td + "/l1") if td else None)
    LAST_RESULTS.append(res1)

    outN = np.concatenate([res1.results[c]["outN"] for c in range(8)], 0)
    attnT_g = (outN[:, :HD, :] / outN[:, HD:HD + 1, :]).reshape(DIM, S)
    attnT = np.empty_like(attnT_g)
    attnT[:, perm] = attnT_g

    nc2 = _get("l2", _build_launch2)
    if os.environ.get("KERNEL_L2_BF16", "1") == "1":
        import ml_dtypes
        l2dt = ml_dtypes.bfloat16
    else:
        l2dt = np.float32
    in2 = []
    for c in range(8):
        i, j = divmod(c, 2)
        in2.append({
            "attnT": np.ascontiguousarray(
                attnT[:, i * 1024:(i + 1) * 1024].astype(l2dt)),
            "woj": np.ascontiguousarray(
                wo[:, j * 1152:(j + 1) * 1152].astype(l2dt)),
            "boj": np.ascontiguousarray(bo[j * 1152:(j + 1) * 1152]).reshape(1, 1152),
        })
    if td:
        os.makedirs(td + "/l2", exist_ok=True)
        for f in os.listdir(td + "/l2"):
            os.unlink(td + "/l2/" + f)
    res2 = bass_utils.run_bass_kernel_spmd(
        nc2, in2, core_ids=list(range(8)),
        tmpdir=(td + "/l2") if td else None)
    LAST_RESULTS.append(res2)

    out = np.empty((S, DIM), np.float32)
    for c in range(8):
        i, j = divmod(c, 2)
        out[i * 1024:(i + 1) * 1024, j * 1152:(j + 1) * 1152] = \
            res2.results[c]["out"].T
    return out[None]
